# revision 8
# baseline (speedup 1.0000x reference)
"""3-level Haar DWT feature kernel for Trainium2 (8 NeuronCores, data-parallel).

Full input x: [256, 131072] f32. Output: [256, 131072] f32 =
concat([cA3, cD3, cD2, cD1], axis=1) per row (pywt wavedec order).

Sharding: batch dim split 8 ways (32 rows per core), no cross-core comm.

Per-core layout trick: a group of R rows is loaded as one [128, F] SBUF tile
where partitions p = r_local*(128/R) + p_sub and row element index
i = p_sub*F + f.  Haar pairs (2k, 2k+1) are then adjacent along the free dim
within one partition at every level, so each level is two stride-2
tensor_tensor ops on the DVE (sum for next cA, diff for cD); the 1/sqrt(2)^l
scales ride on the ScalarEngine (activation copy-with-scale), which is
otherwise idle.  Every output segment is a contiguous [128, n] -> HBM DMA
with >=2KiB contiguous chunks per descriptor.

Deferred scaling: s_l = unscaled pairwise sums; cD_l = (s_{l-1}e - s_{l-1}o)
* c^l, cA_3 = s_3 * c^3.
"""

import numpy as np

import concourse.bacc as bacc
import concourse.bass as bass
import concourse.mybir as mybir
from concourse.tile import TileContext
from concourse.bass_utils import run_bass_kernel_spmd

INV_SQRT2 = 0.7071067811865476
C1 = INV_SQRT2          # cD1 scale
C2 = 0.5                # cD2 scale
C3 = 0.5 * INV_SQRT2    # cA3 / cD3 scale

N_CORES = 8
B, L = 256, 131072
ROWS = B // N_CORES     # 32 rows per core
R = 4                   # rows per group/tile
P_SUB = 128 // R        # 32 partitions per row
F = (L * R) // 128      # 4096 free elems per partition
N_GROUPS = ROWS // R    # 8

FP32 = mybir.dt.float32


def _pairs(ap):
    """[128, N] AP -> (even, odd) stride-2 APs of shape [128, N//2]."""
    p3 = ap.rearrange("p (n two) -> p n two", two=2)
    return p3[:, :, 0], p3[:, :, 1]


def _build_bass():
    nc = bacc.Bacc(
        "TRN2",
        target_bir_lowering=False,
        debug=False,
        num_devices=N_CORES,
    )
    x = nc.dram_tensor("x", [ROWS, L], FP32, kind="ExternalInput")
    out = nc.dram_tensor("out", [ROWS, L], FP32, kind="ExternalOutput")

    with TileContext(nc) as tc:
        with (
            tc.tile_pool(name="xin", bufs=3) as xin_pool,
            tc.tile_pool(name="mid", bufs=2) as mid_pool,
            tc.tile_pool(name="outs", bufs=3) as out_pool,
        ):
            for g in range(N_GROUPS):
                rows = slice(g * R, (g + 1) * R)
                xt = xin_pool.tile([128, F], FP32, tag="xt")
                nc.sync.dma_start(
                    out=xt[:],
                    in_=x[rows].rearrange("r (p f) -> (r p) f", p=P_SUB),
                )

                def store(tile, seg_lo, seg_hi):
                    nc.sync.dma_start(
                        out=out[rows, seg_lo:seg_hi].rearrange(
                            "r (p f) -> r p f", p=P_SUB),
                        in_=tile[:],
                    )

                def level(src_ap, n_out, tag, cd_scale, cd_seg,
                          ca_scale=None, ca_seg=None):
                    """One DWT level: cD=(e-o)*cd_scale stored; returns cA."""
                    ev, od = _pairs(src_ap)
                    du = mid_pool.tile([128, n_out], FP32, tag=f"du{tag}")
                    nc.vector.tensor_tensor(
                        out=du[:], in0=ev, in1=od, op=mybir.AluOpType.subtract)
                    d = out_pool.tile([128, n_out], FP32, tag=f"d{tag}")
                    nc.scalar.mul(d[:], du[:], cd_scale)
                    store(d, *cd_seg)

                    if ca_seg is None:
                        s = mid_pool.tile([128, n_out], FP32, tag=f"s{tag}")
                        nc.vector.tensor_tensor(
                            out=s[:], in0=ev, in1=od, op=mybir.AluOpType.add)
                        return s
                    su = mid_pool.tile([128, n_out], FP32, tag=f"su{tag}")
                    nc.vector.tensor_tensor(
                        out=su[:], in0=ev, in1=od, op=mybir.AluOpType.add)
                    a = out_pool.tile([128, n_out], FP32, tag="a3")
                    nc.scalar.mul(a[:], su[:], ca_scale)
                    store(a, *ca_seg)
                    return a

                # row segments: [cA3 | cD3 | cD2 | cD1] = [0:L/8|L/8:L/4|L/4:L/2|L/2:L]
                s1 = level(xt[:], F // 2, "1", C1, (L // 2, L))
                s2 = level(s1[:], F // 4, "2", C2, (L // 4, L // 2))
                level(s2[:], F // 8, "3", C3, (L // 8, L // 4),
                      ca_scale=C3, ca_seg=(0, L // 8))
    nc.compile()
    return nc


_NC_CACHE = None


def _get_nc():
    global _NC_CACHE
    if _NC_CACHE is None:
        _NC_CACHE = _build_bass()
    return _NC_CACHE


def run_sharded(x, **kwargs):
    """Run on 8 cores; returns (full_output, BassKernelResults)."""
    x = np.ascontiguousarray(np.asarray(x), dtype=np.float32)
    assert x.shape == (B, L), x.shape
    nc = _get_nc()
    in_maps = [
        {"x": np.ascontiguousarray(x[i * ROWS:(i + 1) * ROWS])}
        for i in range(N_CORES)
    ]
    res = run_bass_kernel_spmd(nc, in_maps, list(range(N_CORES)), **kwargs)
    full = np.concatenate([res.results[i]["out"] for i in range(N_CORES)], axis=0)
    return full, res


def kernel(x):
    out, _ = run_sharded(x)
    return out


# revision 12
# speedup vs baseline: 37.2094x; 37.2094x over previous
"""3-level Haar DWT feature kernel for Trainium2 (8 NeuronCores, data-parallel).

Full input x: [256, 131072] f32. Output: [256, 131072] f32 =
concat([cA3, cD3, cD2, cD1], axis=1) per row (pywt wavedec order).

Sharding: batch dim split 8 ways (32 rows per core), no cross-core comm.

Per-core layout trick: a group of R rows is loaded as one [128, F] SBUF tile
where partitions p = r_local*(128/R) + p_sub and row element index
i = p_sub*F + f.  Haar pairs (2k, 2k+1) are then adjacent along the free dim
within one partition at every level, so each level is two stride-2
tensor_tensor ops on the DVE (sum for next cA, diff for cD); the 1/sqrt(2)^l
scales ride on the ScalarEngine (activation copy-with-scale), which is
otherwise idle.  Every output segment is a contiguous [128, n] -> HBM DMA
with >=4KiB contiguous chunks per descriptor (R=8 rows/group doubles chunk
sizes vs R=4; <4KiB descriptors force internal read-modify-write in HBM).

Loads are issued from the SP sequencer (qSPDynamicHW ring) and stores from
the ACT sequencer (qActDynamicHW ring): TRN2 has two hardware DGE rings and
DMAs on one ring drain in FIFO order, so putting the 16MiB/core of loads and
16MiB/core of stores on separate rings lets them stream concurrently
(measured ~2.7x faster than a single ring).

Deferred scaling: s_l = unscaled pairwise sums; cD_l = (s_{l-1}e - s_{l-1}o)
* c^l, cA_3 = s_3 * c^3.
"""

import numpy as np

import concourse.bacc as bacc
import concourse.bass as bass
import concourse.mybir as mybir
from concourse.tile import TileContext
from concourse.bass_utils import run_bass_kernel_spmd

INV_SQRT2 = 0.7071067811865476
C1 = INV_SQRT2          # cD1 scale
C2 = 0.5                # cD2 scale
C3 = 0.5 * INV_SQRT2    # cA3 / cD3 scale

N_CORES = 8
B, L = 256, 131072
ROWS = B // N_CORES     # 32 rows per core
R = 8                   # rows per group/tile
P_SUB = 128 // R        # 16 partitions per row
F = (L * R) // 128      # 8192 free elems per partition
N_GROUPS = ROWS // R    # 4

FP32 = mybir.dt.float32


def _pairs(ap):
    """[128, N] AP -> (even, odd) stride-2 APs of shape [128, N//2]."""
    p3 = ap.rearrange("p (n two) -> p n two", two=2)
    return p3[:, :, 0], p3[:, :, 1]


def _build_bass():
    nc = bacc.Bacc(
        "TRN2",
        target_bir_lowering=False,
        debug=False,
        num_devices=N_CORES,
    )
    x = nc.dram_tensor("x", [ROWS, L], FP32, kind="ExternalInput")
    out = nc.dram_tensor("out", [ROWS, L], FP32, kind="ExternalOutput")

    with TileContext(nc) as tc:
        with (
            tc.tile_pool(name="xin", bufs=2) as xin_pool,
            tc.tile_pool(name="mid", bufs=1) as mid_pool,
            tc.tile_pool(name="outs", bufs=2) as out_pool,
        ):
            for g in range(N_GROUPS):
                rows = slice(g * R, (g + 1) * R)
                xt = xin_pool.tile([128, F], FP32, tag="xt")
                nc.sync.dma_start(
                    out=xt[:],
                    in_=x[rows].rearrange("r (p f) -> (r p) f", p=P_SUB),
                )

                def store(tile, seg_lo, seg_hi):
                    nc.scalar.dma_start(
                        out=out[rows, seg_lo:seg_hi].rearrange(
                            "r (p f) -> r p f", p=P_SUB),
                        in_=tile[:],
                    )

                def level(src_ap, n_out, tag, cd_scale, cd_seg,
                          ca_scale=None, ca_seg=None):
                    """One DWT level: cD=(e-o)*cd_scale stored; returns cA."""
                    ev, od = _pairs(src_ap)
                    du = mid_pool.tile([128, n_out], FP32, tag=f"du{tag}")
                    nc.vector.tensor_tensor(
                        out=du[:], in0=ev, in1=od, op=mybir.AluOpType.subtract)
                    d = out_pool.tile([128, n_out], FP32, tag=f"d{tag}")
                    nc.scalar.mul(d[:], du[:], cd_scale)
                    store(d, *cd_seg)

                    if ca_seg is None:
                        s = mid_pool.tile([128, n_out], FP32, tag=f"s{tag}")
                        nc.vector.tensor_tensor(
                            out=s[:], in0=ev, in1=od, op=mybir.AluOpType.add)
                        return s
                    su = mid_pool.tile([128, n_out], FP32, tag=f"su{tag}")
                    nc.vector.tensor_tensor(
                        out=su[:], in0=ev, in1=od, op=mybir.AluOpType.add)
                    a = out_pool.tile([128, n_out], FP32, tag="a3")
                    nc.scalar.mul(a[:], su[:], ca_scale)
                    store(a, *ca_seg)
                    return a

                # row segments: [cA3 | cD3 | cD2 | cD1] = [0:L/8|L/8:L/4|L/4:L/2|L/2:L]
                s1 = level(xt[:], F // 2, "1", C1, (L // 2, L))
                s2 = level(s1[:], F // 4, "2", C2, (L // 4, L // 2))
                level(s2[:], F // 8, "3", C3, (L // 8, L // 4),
                      ca_scale=C3, ca_seg=(0, L // 8))
    nc.compile()
    return nc


_NC_CACHE = None


def _get_nc():
    global _NC_CACHE
    if _NC_CACHE is None:
        _NC_CACHE = _build_bass()
    return _NC_CACHE


def run_sharded(x, **kwargs):
    """Run on 8 cores; returns (full_output, BassKernelResults)."""
    x = np.ascontiguousarray(np.asarray(x), dtype=np.float32)
    assert x.shape == (B, L), x.shape
    nc = _get_nc()
    in_maps = [
        {"x": np.ascontiguousarray(x[i * ROWS:(i + 1) * ROWS])}
        for i in range(N_CORES)
    ]
    res = run_bass_kernel_spmd(nc, in_maps, list(range(N_CORES)), **kwargs)
    full = np.concatenate([res.results[i]["out"] for i in range(N_CORES)], axis=0)
    return full, res


def kernel(x):
    out, _ = run_sharded(x)
    return out
